# revision 42
# baseline (speedup 1.0000x reference)
"""Trainium2 Bass kernel for a 16-head dense attention block (B=1, S=2048, D=2048).

Sharding: 2 heads per core across 8 cores (tensor parallel on heads).
The reference's (deliberate) transpose(2,3)+reshape before the output
projection makes output rows [h*128:(h+1)*128) depend ONLY on head h, so
per-core outputs are disjoint row blocks -> host-side concat, no collectives.

v3 design: single software-pipelined instruction stream. Attention for
q-group g is woven between the QKV-projection matmul chains of group g+1
(attention for the last group weaves into the first output-projection
chain), so the ScalarE exp stream -- the attention-phase bottleneck --
runs entirely under PE's projection matmuls. Everything is bf16 (halves
HBM traffic, full PE rate at any tile width), O is computed directly in
[q, dh] layout (pt chunks stationary -> no epilogue transposes), row
sums Z are [128,1] matmuls (rhs=ones) accumulated in PSUM alongside O
(ap_size=1 -> free on PE), the causal mask is one [128,128] 0/1
triangle multiplied into pt on the Pool engine (no mask DMA), and 1/Z
is fused into the PSUM->SBUF copy of O.

Engine budget per core (cost model): PE ~139us busy (the floor for this
sharding at bf16: QKV 82 + attention 29 + proj 27), ACT ~49us (exp),
DVE ~25us (rope + epilogue), Pool ~31us (PSUM->SBUF copies, tri-mul),
DMA ~73us (26MB at 360GB/s).
"""

import math

import numpy as np

S = 2048
D = 2048
H = 16
DH = 128
N_CORES = 8
HPC = H // N_CORES          # heads per core
NH = HPC * DH               # per-core head rows (256)
P = 128
QG = 512                    # q/s-group width
NQG = S // QG               # 4
NKT = S // P                # 16 k tiles
NDT = D // P                # 16 d tiles

SKIP, ZERO, TRI, GEN = 0, 1, 2, 3

_CACHE = {}


def _build(block_kind):
    """block_kind: tuple of NQG tuples of NKT (kind, arg) pairs."""
    import contextlib
    import os
    import concourse.tile as tile
    from concourse import bacc, mybir

    B = lambda k, d: int(os.environ.get(k, d))
    f32 = mybir.dt.float32
    bf16 = mybir.dt.bfloat16
    EXP = mybir.ActivationFunctionType.Exp
    SCL = 1.0 / math.sqrt(DH)

    any_gen = any(k == GEN for row in block_kind for k, _ in row)

    nc = bacc.Bacc("TRN2", target_bir_lowering=False, debug=False,
                   num_devices=N_CORES)

    xT = nc.dram_tensor("xT", [D, S], bf16, kind="ExternalInput").ap()
    wqT = nc.dram_tensor("wqT", [D, NH], bf16, kind="ExternalInput").ap()
    wkT = nc.dram_tensor("wkT", [D, NH], bf16, kind="ExternalInput").ap()
    wvT = nc.dram_tensor("wvT", [D, NH], bf16, kind="ExternalInput").ap()
    woT = nc.dram_tensor("woT", [S, D], bf16, kind="ExternalInput").ap()
    ck = nc.dram_tensor("ck", [DH, S], bf16, kind="ExternalInput").ap()
    sk = nc.dram_tensor("sk", [DH, S], bf16, kind="ExternalInput").ap()
    tri = nc.dram_tensor("tri", [P, P], bf16, kind="ExternalInput").ap()
    psw = nc.dram_tensor("psw", [P, P], bf16, kind="ExternalInput").ap()
    ones = nc.dram_tensor("ones", [P, 1], bf16, kind="ExternalInput").ap()
    if any_gen:
        maskT = nc.dram_tensor("maskT", [S, S], bf16,
                               kind="ExternalInput").ap()
        maskT_v = maskT.rearrange("(t p) s -> t p s", p=P)
    out = nc.dram_tensor("out", [NH, D], f32, kind="ExternalOutput").ap()

    xT_v = xT.rearrange("(t p) s -> t p s", p=P)           # [16,128,S]
    wT_v = {"q": wqT.rearrange("(t p) n -> t p n", p=P),
            "k": wkT.rearrange("(t p) n -> t p n", p=P),
            "v": wvT.rearrange("(t p) n -> t p n", p=P)}
    woT_v = woT.rearrange("(t p) m -> t p m", p=P)

    with tile.TileContext(nc) as tc:
        with contextlib.ExitStack() as stack:
            consts = stack.enter_context(tc.tile_pool(name="consts", bufs=1))
            qkv = stack.enter_context(tc.tile_pool(name="qkv", bufs=1))
            proj_sb = stack.enter_context(
                tc.tile_pool(name="proj_sb", bufs=1))
            rope_sb = stack.enter_context(
                tc.tile_pool(name="rope_sb", bufs=B("BR", 3)))
            att_sb = stack.enter_context(
                tc.tile_pool(name="att_sb", bufs=B("BA", 6)))
            eps_sb = stack.enter_context(
                tc.tile_pool(name="eps_sb", bufs=B("BE", 2)))
            wo_sb = stack.enter_context(tc.tile_pool(name="wo_sb", bufs=1))
            r_sb = stack.enter_context(tc.tile_pool(name="r_sb", bufs=4))

            qt = [qkv.tile([P, S], bf16, tag=f"qt{h}", name=f"qt{h}")
                  for h in range(HPC)]
            kt_ = [qkv.tile([P, S], bf16, tag=f"kt{h}", name=f"kt{h}")
                   for h in range(HPC)]
            vt = qkv.tile([P, NKT, NH], bf16, tag="v")   # [k-part, ktile, n]
            o_sb = [qkv.tile([P, NKT, P], bf16, tag=f"o{h}", name=f"o{h}")
                    for h in range(HPC)]                 # [q-part, jt, dh]

            # weights + rope consts on the scalar queue, first-use order
            wts = {}
            rope_t = {}

            def load_w(kind, nchunk=1):
                t = proj_sb.tile([P, NDT, NH], bf16, tag=f"w{kind}",
                                 name=f"w{kind}")
                step = NDT // nchunk
                for c in range(nchunk):
                    cs = slice(c * step, (c + 1) * step)
                    nc.scalar.dma_start(
                        t[:, cs], wT_v[kind][cs].rearrange("t p n -> p t n"))
                wts[kind] = t

            def load_rope(nm, src):
                t = proj_sb.tile([DH, S], bf16, tag=nm, name=nm)
                nc.scalar.dma_start(t[:], src[:])
                rope_t[nm] = t

            load_w("q", nchunk=4)
            psw_t = consts.tile([P, P], bf16, tag="psw")
            nc.scalar.dma_start(psw_t[:], psw[:])
            load_rope("ck", ck)
            load_rope("sk", sk)
            load_w("k")
            load_w("v")
            tri_t = consts.tile([P, P], bf16, tag="tri")
            nc.scalar.dma_start(tri_t[:], tri[:])
            ones_t = consts.tile([P, 1], bf16, tag="ones")
            nc.scalar.dma_start(ones_t[:], ones[:])

            # PSUM pools: ps_sc 2 + ps_o 2 + ps_z 1 + ps_q 2 + ps_v 1 = 8.
            # att pools opened first so the qkv pools can pop (LIFO) when
            # the output projection needs its banks.
            att_ps = (tc.tile_pool(name="ps_sc", bufs=B("BS", 2),
                                   space="PSUM"),
                      tc.tile_pool(name="ps_o", bufs=B("BO", 1),
                                   space="PSUM"),
                      tc.tile_pool(name="ps_z", bufs=B("BZ", 1),
                                   space="PSUM"))
            ps_sc, ps_o, ps_z = [c.__enter__() for c in att_ps]
            qkv_ps = (tc.tile_pool(name="ps_q", bufs=B("BQ", 2),
                                   space="PSUM"),
                      tc.tile_pool(name="ps_v", bufs=B("BV", 1),
                                   space="PSUM"))
            ps_q, ps_v = [c.__enter__() for c in qkv_ps]

            xs_tiles = {}

            def emit_xs_loads(g):
                xs_c = []
                for c in range(4):
                    cs = slice(c * 4, c * 4 + 4)
                    xc = proj_sb.tile([P, 4, QG], bf16, tag=f"xs{c}",
                                      bufs=B("BX", 3), name=f"xs{c}")
                    if g == 0 and c == 0:
                        for hh in range(2):
                            hs = slice(hh * 2, hh * 2 + 2)
                            nc.sync.dma_start(
                                xc[:, hs],
                                xT_v[hs, :, 0:QG].rearrange("t p s -> p t s"))
                    else:
                        nc.sync.dma_start(
                            xc[:],
                            xT_v[cs, :, g * QG:(g + 1) * QG].rearrange(
                                "t p s -> p t s"))
                    xs_c.append(xc)
                xs_tiles[g] = xs_c

            def xs(g, dt, lsl=slice(None)):
                return xs_tiles[g][dt // 4][:, dt % 4, lsl]

            def qk_pieces(g, kind, split0=False):
                """8 pieces (2 heads x 4): one 16-matmul chain per head plus
                a PSUM->SBUF copy (Pool) at the end of each chain. The rope
                finish (PE pair-swap matmul reusing the same PSUM bank, a
                second Pool copy, three DVE elementwise ops) is emitted as a
                separate weave item (rope_fin) a couple of pieces later."""
                st_ = {}
                pieces = []
                dst = qt if kind == "q" else kt_
                sl = slice(g * QG, (g + 1) * QG)

                def piece(h, j):
                    if j == 0:
                        st_[h] = ps_q.tile([P, QG], f32, tag="pq", name="pq")
                    ps = st_[h]
                    for dt in range(4 * j, 4 * j + 4):
                        nc.tensor.matmul(ps[:],
                                         wts[kind][:, dt, h * P:(h + 1) * P],
                                         xs(g, dt), start=(dt == 0),
                                         stop=(dt == NDT - 1))
                    if j == 3:
                        nc.vector.tensor_copy(dst[h][:, sl], ps[:])

                def piece2(h, j2):
                    if j2 == 0:
                        st_[h] = ps_q.tile([P, QG], f32, tag="pq", name="pq")
                    ps = st_[h]
                    for dt in range(2 * j2, 2 * j2 + 2):
                        nc.tensor.matmul(ps[:],
                                         wts[kind][:, dt, h * P:(h + 1) * P],
                                         xs(g, dt), start=(dt == 0),
                                         stop=(dt == NDT - 1))
                    if j2 == 7:
                        nc.vector.tensor_copy(dst[h][:, sl], ps[:])

                for h in range(HPC):
                    if split0 and h == 0:
                        for j2 in range(8):
                            pieces.append((lambda j2=j2: piece2(0, j2)))
                    else:
                        for j in range(4):
                            pieces.append((lambda h=h, j=j: piece(h, j)))
                return pieces, st_

            def rope_fin(g, kind, h, st_):
                """Pair-swap on PE (reuses the chain's PSUM bank), then
                q'/k' = C*m + S*swap(m) on DVE."""
                dst = qt if kind == "q" else kt_
                sl = slice(g * QG, (g + 1) * QG)
                m = dst[h]
                ps = st_[h]
                nc.tensor.matmul(ps[:], psw_t[:], m[:, sl],
                                 start=True, stop=True)
                sw = rope_sb.tile([P, QG], bf16, tag="sw", name="sw")
                nc.vector.tensor_copy(sw[:], ps[:])
                t1 = rope_sb.tile([P, QG], bf16, tag="t1", name="t1")
                nc.vector.tensor_mul(t1[:], m[:, sl], rope_t["ck"][:, sl])
                nc.vector.tensor_mul(sw[:], sw[:], rope_t["sk"][:, sl])
                nc.vector.tensor_add(m[:, sl], t1[:], sw[:])

            def v_pieces(g):
                """8 pieces (4 s-tiles x 2): V projection chains."""
                st_ = {}
                pieces = []

                def piece(st, j):
                    if j == 0:
                        st_[st] = ps_v.tile([P, NH], f32, tag="pv", name="pv")
                    ps = st_[st]
                    lsl = slice((st % 4) * P, (st % 4) * P + P)
                    for dt in range(8 * j, 8 * j + 8):
                        nc.tensor.matmul(ps[:], xs(g, dt, lsl),
                                         wts["v"][:, dt], start=(dt == 0),
                                         stop=(dt == NDT - 1))
                    if j == 1:
                        nc.vector.tensor_copy(vt[:, st], ps[:])

                for st in range(g * 4, g * 4 + 4):
                    for j in range(2):
                        pieces.append((lambda st=st, j=j: piece(st, j)))
                return pieces

            # ---------------- attention item machinery -------------------
            att_state = {}

            def att_setup(g):
                kinds = block_kind[g]
                active = [kt for kt in range(NKT) if kinds[kt][0] != SKIP]

                def c_first(c):
                    for kt in active:
                        k, a = kinds[kt]
                        if k == TRI and c < a:
                            continue
                        return kt

                def c_last(c):
                    for kt in reversed(active):
                        k, a = kinds[kt]
                        if k == TRI and c < a:
                            continue
                        return kt

                att_state[g] = dict(
                    kinds=kinds, active=active, c_first=c_first,
                    c_last=c_last,
                    pso=[ps_o.tile([P, 4, P], f32, tag=f"pso{h}",
                                   name=f"pso{h}") for h in range(HPC)],
                    psz=ps_z.tile([P, HPC, 4], f32, tag="psz", name="psz"),
                    pt={})

            def att_A(g, kt):
                st = att_state[g]
                kind, arg = st["kinds"][kt]
                off = arg * P if kind == TRI else 0
                osl = slice(off, QG)
                qsl = slice(g * QG + off, (g + 1) * QG)
                ksl = slice(kt * P, (kt + 1) * P)
                if kind == GEN:
                    mt = att_sb.tile([P, QG], bf16, tag="mask", name="mt")
                    nc.sync.dma_start(
                        mt[:], maskT_v[kt][:, g * QG:(g + 1) * QG])
                for h in range(HPC):
                    psc = ps_sc.tile([P, QG], f32, tag="sc", name="sc")
                    nc.tensor.matmul(psc[:, osl], kt_[h][:, ksl],
                                     qt[h][:, qsl], start=True, stop=True)
                    pt = att_sb.tile([P, QG], bf16, tag="pt", name="pt")
                    if kind == GEN:
                        sm = att_sb.tile([P, QG], f32, tag="sm", name="sm")
                        nc.vector.tensor_add(sm[:], psc[:], mt[:])
                        nc.scalar.activation(pt[:], sm[:], EXP,
                                             scale=SCL)
                    else:
                        nc.scalar.activation(pt[:, osl], psc[:, osl], EXP,
                                             scale=SCL)
                    if kind == TRI:
                        tsl = slice(arg * P, arg * P + P)
                        nc.gpsimd.tensor_mul(pt[:, tsl], pt[:, tsl],
                                             tri_t[:])
                    st["pt"][(kt, h)] = pt

            def att_B(g, kt):
                # PSUM accumulation groups are per 2KB zero-region (bank):
                # start=True zeroes the WHOLE bank and only one group may be
                # open per bank. pso[h] packs 4 q-chunks in one bank and psz
                # packs both heads' Z columns in one bank, so each bank gets
                # exactly one start (first write) and one stop (last write).
                st = att_state[g]
                kind, arg = st["kinds"][kt]
                c0 = arg if kind == TRI else 0
                kt_first, kt_last = st["active"][0], st["active"][-1]
                kf, af = st["kinds"][kt_first]
                c_first_overall = af if kf == TRI else 0
                for h in range(HPC):
                    pt = st["pt"].pop((kt, h))
                    for c in range(c0, 4):
                        csl = slice(c * P, (c + 1) * P)
                        first = kt == kt_first and c == c_first_overall
                        last = kt == kt_last and c == 3
                        nc.tensor.matmul(
                            st["pso"][h][:, c, :], pt[:, csl],
                            vt[:, kt, h * P:(h + 1) * P],
                            start=first, stop=last)
                        nc.tensor.matmul(
                            st["psz"][:, h, c:c + 1], pt[:, csl], ones_t[:],
                            start=(first and h == 0),
                            stop=(last and h == HPC - 1))

            def att_epilogue(g):
                st = att_state[g]
                for h in range(HPC):
                    rt = eps_sb.tile([P, 4], f32, tag="rt", name="rt")
                    nc.vector.reciprocal(rt[:], st["psz"][:, h, :])
                    for c in range(4):
                        nc.scalar.mul(o_sb[h][:, g * 4 + c],
                                      st["pso"][h][:, c, :], rt[:, c:c + 1])

            def att_items(g):
                """(min_piece, thunk) list. min_piece encodes LATENCY, not
                just dependency: the k-rope of group g completes ~5us (6
                pieces) after the k-chain matmuls, so diagonal-block scores
                are held until piece ~14; B trails its A by 2 pieces (exp
                latency) and diag B additionally trails its v-chain."""
                st = att_state[g]
                kinds = st["kinds"]
                n0 = sum(1 for kt in st["active"] if kt < 4 * g)
                items = []
                last_b = 0
                j = 0
                for kt in st["active"]:
                    kind, arg = kinds[kt]
                    if kt < 4 * g:       # full block, no g-local deps
                        a_min = 4 + j
                        b_min = a_min + 2
                        j += 1
                    else:                # diagonal: k(g) rope + vt[:, kt]
                        i = kt - 4 * g
                        a_min = max(15 + i, 4 + n0 + i)
                        b_min = max(a_min + 1, 11 + 2 * i)
                    items.append((a_min, lambda g=g, kt=kt: att_A(g, kt)))
                    items.append((b_min, lambda g=g, kt=kt: att_B(g, kt)))
                    last_b = max(last_b, b_min)
                items.append((last_b + 1, lambda g=g: att_epilogue(g)))
                return items

            def weave(pieces, items):
                """Emit pieces in order; after each piece emit every queued
                item whose min_piece has been reached (queue order)."""
                queue = list(items)
                for idx, piece in enumerate(pieces):
                    piece()
                    qi = 0
                    while qi < len(queue):
                        mp, t = queue[qi]
                        if mp <= idx + 1:
                            queue.pop(qi)
                            t()
                        else:
                            qi += 1
                for mp, t in queue:
                    t()

            # ---------------- the pipelined stream -----------------------
            emit_xs_loads(0)
            q0_pieces, q0_st = qk_pieces(0, "q", split0=True)
            weave(q0_pieces, [(11, lambda: rope_fin(0, "q", 0, q0_st))])
            carry = [(2, lambda: rope_fin(0, "q", 1, q0_st))]

            wo_t = {}
            psr = {}

            def proj_part1_pieces():
                """mg0 jt0..11 accumulation, 12 pieces of 2 matmuls."""
                pieces = []

                def setup():
                    for jt in range(NKT):
                        t = wo_sb.tile([P, S], bf16, tag=f"wo{jt}",
                                       name=f"wo{jt}")
                        nc.sync.dma_start(t[:], woT_v[jt][:])
                        wo_t[jt] = t
                    for h in range(HPC):
                        psr[(0, h)] = ps_r.tile([P, QG], f32, tag="psr",
                                                name="psr")

                def piece(jt):
                    if jt == 0:
                        setup()
                    for h in range(HPC):
                        nc.tensor.matmul(psr[(0, h)][:], o_sb[h][:, jt],
                                         wo_t[jt][:, 0:QG],
                                         start=(jt == 0), stop=(jt == NKT - 1))

                for jt in range(12):
                    pieces.append((lambda jt=jt: piece(jt)))
                return pieces

            ps_r = None
            ps_r_ctx = None

            def pool_swap():
                # close QKV PSUM pools; open proj pool (3 banks)
                nonlocal ps_r, ps_r_ctx
                for c in reversed(qkv_ps):
                    c.__exit__(None, None, None)
                ps_r_ctx = tc.tile_pool(name="ps_r", bufs=B("BP", 3),
                                        space="PSUM")
                ps_r = ps_r_ctx.__enter__()

            # the interleaved schedule assumes attention(g) only reads
            # k/v tiles produced by groups <= g (true for causal-style
            # masks). Anything else takes the sequential fallback.
            causal_ok = all(
                kt <= 4 * g + 3
                for g, row in enumerate(block_kind)
                for kt, (k, _) in enumerate(row) if k != SKIP)

            if causal_ok:
                for g in range(NQG):
                    att_setup(g)
                    if g + 1 < NQG:
                        emit_xs_loads(g + 1)
                    k_pieces, k_st = qk_pieces(g, "k")
                    cover = k_pieces + v_pieces(g)
                    rope_items = carry + [
                        (9, lambda g=g, st=k_st: rope_fin(g, "k", 0, st)),
                        (12, lambda g=g, st=k_st: rope_fin(g, "k", 1, st))]
                    carry = []
                    if g + 1 < NQG:
                        qn_pieces, qn_st = qk_pieces(g + 1, "q")
                        cover += qn_pieces
                        rope_items += [
                            (23, lambda g=g, st=qn_st: rope_fin(
                                g + 1, "q", 0, st))]
                        carry = [(2, lambda g=g, st=qn_st: rope_fin(
                            g + 1, "q", 1, st))]
                    else:
                        cover += [pool_swap] + proj_part1_pieces()
                    weave(cover, sorted(att_items(g) + rope_items,
                                        key=lambda it: it[0]))
            else:
                # sequential: all QKV -> all attention -> projection
                for mp, t in carry:
                    t()
                for g in range(NQG):
                    if g + 1 < NQG:
                        emit_xs_loads(g + 1)
                    k_pieces, k_st = qk_pieces(g, "k")
                    for p in k_pieces:
                        p()
                    rope_fin(g, "k", 0, k_st)
                    rope_fin(g, "k", 1, k_st)
                    for p in v_pieces(g):
                        p()
                    if g + 1 < NQG:
                        qn_pieces, qn_st = qk_pieces(g + 1, "q")
                        for p in qn_pieces:
                            p()
                        rope_fin(g + 1, "q", 0, qn_st)
                        rope_fin(g + 1, "q", 1, qn_st)
                for g in range(NQG):
                    att_setup(g)
                    for kt in att_state[g]["active"]:
                        att_A(g, kt)
                        att_B(g, kt)
                    att_epilogue(g)
                pool_swap()
                for p in proj_part1_pieces():
                    p()

            # ---------------- remaining output projection ----------------
            def proj_finish(mg, jt_from):
                for jt in range(jt_from, NKT):
                    msl = slice(mg * QG, (mg + 1) * QG)
                    for h in range(HPC):
                        nc.tensor.matmul(psr[(mg, h)][:], o_sb[h][:, jt],
                                         wo_t[jt][:, msl],
                                         start=(jt == 0), stop=(jt == NKT - 1))
                msl = slice(mg * QG, (mg + 1) * QG)
                for h in range(HPC):
                    rs = r_sb.tile([P, QG], f32, tag="rs", name="rs")
                    # h0 on DVE, h1 on ACT so the two drain copies overlap
                    if h == 0:
                        nc.vector.tensor_copy(rs[:], psr[(mg, h)][:])
                    else:
                        nc.scalar.copy(rs[:], psr[(mg, h)][:])
                    nc.sync.dma_start(out[h * P:(h + 1) * P, msl], rs[:])

            proj_finish(0, 12)
            for mg in range(1, NQG):
                for h in range(HPC):
                    psr[(mg, h)] = ps_r.tile([P, QG], f32, tag="psr",
                                             name="psr")
                proj_finish(mg, 0)

            ps_r_ctx.__exit__(None, None, None)
            for c in reversed(att_ps):
                c.__exit__(None, None, None)

    nc.compile()
    return nc


def _classify_mask(mask):
    """Per (g, kt) block of mask^T [S(k), S(q)]: SKIP if exp underflows to 0
    for the whole block, ZERO if exactly zero, TRI(i) if it is the canonical
    causal diagonal block, else GEN."""
    maskT = mask.T
    kinds = []
    for g in range(NQG):
        row = []
        for kt in range(NKT):
            blk = maskT[kt * P:(kt + 1) * P, g * QG:(g + 1) * QG]
            if np.all(blk <= -1e5):
                row.append((SKIP, 0))
            elif not blk.any():
                row.append((ZERO, 0))
            else:
                i = kt - 4 * g
                if 0 <= i < 4:
                    ref = np.zeros((P, QG), np.float32)
                    kk = np.arange(P)[:, None]
                    qq = np.arange(QG)[None, :]
                    ref[(kk + kt * P) > (qq + g * QG)] = blk.min()
                    if np.array_equal(blk, ref) and np.all(blk.min() <= -1e5):
                        row.append((TRI, i))
                        continue
                row.append((GEN, 0))
        kinds.append(tuple(row))
    return tuple(kinds)


def _get_nc(block_kind):
    key = ("nc", block_kind)
    if key not in _CACHE:
        _CACHE[key] = _build(block_kind)
    return _CACHE[key]


def _prep_inputs(x, freqs_cos, freqs_sin, mask, wq, wk, wv, wo, block_kind):
    import ml_dtypes
    f = np.float32
    bf = ml_dtypes.bfloat16
    x = np.asarray(x, f).reshape(S, D)
    mask = np.asarray(mask, f).reshape(S, S)
    wq, wk, wv, wo = (np.asarray(w, f) for w in (wq, wk, wv, wo))
    cos = np.asarray(freqs_cos, f)
    sin = np.asarray(freqs_sin, f)

    xT = np.ascontiguousarray(x.T.astype(bf))
    woT = np.ascontiguousarray(wo.T.astype(bf))

    C = np.repeat(cos.T, 2, axis=0)          # [128, S], rows 2j,2j+1 = cos_j
    Sg = np.repeat(sin.T, 2, axis=0)
    Sg[0::2] *= -1.0                          # even rows: -sin, odd: +sin
    tri01 = (np.arange(P)[:, None] <= np.arange(P)[None, :]).astype(bf)
    pswap = (np.arange(P)[:, None] == (np.arange(P)[None, :] ^ 1)).astype(bf)
    common = {
        "xT": xT, "woT": woT,
        "ck": np.ascontiguousarray(C.astype(bf)),
        "sk": np.ascontiguousarray(Sg.astype(bf)),
        "tri": tri01,
        "psw": pswap,
        "ones": np.ones((P, 1), bf),
    }
    if any(k == GEN for row in block_kind for k, _ in row):
        common["maskT"] = np.ascontiguousarray(mask.T.astype(bf))
    in_maps = []
    for c in range(N_CORES):
        rows = slice(c * NH, (c + 1) * NH)
        in_maps.append(dict(
            common,
            wqT=np.ascontiguousarray(wq[rows].T.astype(bf)),
            wkT=np.ascontiguousarray(wk[rows].T.astype(bf)),
            wvT=np.ascontiguousarray(wv[rows].T.astype(bf)),
        ))
    return in_maps


def kernel(x, freqs_cos, freqs_sin, mask, wq, wk, wv, wo, start_pos):
    from concourse.bass_utils import run_bass_kernel_spmd

    block_kind = _classify_mask(
        np.asarray(mask, np.float32).reshape(S, S))
    in_maps = _prep_inputs(x, freqs_cos, freqs_sin, mask, wq, wk, wv, wo,
                           block_kind)
    nc = _get_nc(block_kind)
    res = run_bass_kernel_spmd(nc, in_maps, core_ids=list(range(N_CORES)))
    full = np.concatenate([res.results[c]["out"] for c in range(N_CORES)],
                          axis=0)
    return full.reshape(1, S, D).astype(np.float32)
